# revision 4
# baseline (speedup 1.0000x reference)
"""Trainium2 Bass kernel for a GPT-OSS-style MoE MLP block (top-2 of 8 experts).

Strategy (expert-parallel, full_io):
  - Host computes router softmax + top-2 + renormalized combine weights
    (tiny: [2048, 8]); margins between 2nd/3rd affinities are >=2e-5 for the
    target data, far above fp32 noise, so selection matches the reference.
  - Tokens are dispatched per expert (one expert per NeuronCore), padded to a
    common capacity C; each core runs gate/up matmuls, SiLU*up, down matmul in
    bf16 with fp32 accumulation, scales rows by the combine weight on-device.
  - Host gathers the 8 weighted partial outputs and scatter-adds into [T, D].

Layouts (per core, host-prepared):
  tT: [23, 128, C]   bf16  tokens^T, D padded 2880->2944 (=23*128)
  wg: [23, 128, 2944] bf16  w_gate[e], both dims padded to 2944
  wu: [23, 128, 2944] bf16  w_up[e]
  wd: [23, 128, 2880] bf16  w_down[e], I padded to 2944
  wv: [ceil(C/128), 128] f32  combine weights (0 in padding slots)
  y : [C, 2880] f32  output (weighted expert output rows)
"""

import math
import os

import ml_dtypes
import numpy as np

T, D, E, TOPK = 2048, 2880, 8, 2
P = 128
DP = 2944  # D and I padded to 23*128
KD = DP // P  # 23 contraction chunks for gate/up
KI = DP // P  # 23 contraction chunks for down
N_CORES = 8

BF16 = ml_dtypes.bfloat16

_cache = {}


def _route(x, w_router):
    """Host top-2 routing, mirroring the jax reference numerics."""
    t = np.ascontiguousarray(x.reshape(-1, D).astype(np.float32))
    logits = t @ w_router.astype(np.float32)  # [T, E]
    m = logits.max(axis=-1, keepdims=True)
    ex = np.exp(logits - m)
    aff = ex / ex.sum(axis=-1, keepdims=True)
    i1 = aff.argmax(axis=-1)
    a2 = aff.copy()
    a2[np.arange(aff.shape[0]), i1] = -np.inf
    i2 = a2.argmax(axis=-1)
    v1 = aff[np.arange(aff.shape[0]), i1]
    v2 = aff[np.arange(aff.shape[0]), i2]
    s = v1 + v2
    return t, i1, i2, v1 / s, v2 / s


def _blocks(total, max_bs, align):
    """Split `total` into near-equal blocks of size <= max_bs, multiple of
    `align` (except possibly the last)."""
    nb = math.ceil(total / max_bs)
    bs = math.ceil(total / nb / align) * align
    out = []
    off = 0
    while off < total:
        w = min(bs, total - off)
        out.append((off, w))
        off += w
    return out


def _build_program(C):
    import concourse.bacc as bacc
    import concourse.mybir as mybir
    import concourse.tile as tile

    f32 = mybir.dt.float32
    bf16 = mybir.dt.bfloat16

    nCc = math.ceil(C / P)
    c_chunks = _blocks(C, P, 32)          # mm3 output partition chunks
    c_blocks = _blocks(C, 512, 32)        # phase-1 moving free-dim blocks
    d_tiles = _blocks(D, 480, 32)         # 6 x 480
    # group I chunks for weight DMA granularity (2 chunks = 512B runs)
    i_groups = _blocks(KI, 2, 1)

    nc = bacc.Bacc("TRN2", target_bir_lowering=False, debug=False,
                   num_devices=N_CORES)

    tT_d = nc.dram_tensor("tT", [KD, P, C], bf16, kind="ExternalInput").ap()
    wg_d = nc.dram_tensor("wg", [KD, P, DP], bf16, kind="ExternalInput").ap()
    wu_d = nc.dram_tensor("wu", [KD, P, DP], bf16, kind="ExternalInput").ap()
    wd_d = nc.dram_tensor("wd", [KI, P, D], bf16, kind="ExternalInput").ap()
    wv_d = nc.dram_tensor("wv", [nCc, P], f32, kind="ExternalInput").ap()
    y_d = nc.dram_tensor("y", [C, D], f32, kind="ExternalOutput").ap()

    with tile.TileContext(nc) as tc:
        with tc.tile_pool(name="resident", bufs=1) as res_pool, \
             tc.tile_pool(name="wgu", bufs=2) as wgu_pool, \
             tc.tile_pool(name="wd", bufs=2) as wd_pool, \
             tc.tile_pool(name="tmp", bufs=2) as tmp_pool, \
             tc.tile_pool(name="yev", bufs=3) as y_pool:

            tok = res_pool.tile([P, KD, C], bf16, tag="tok")
            nc.sync.dma_start(out=tok, in_=tT_d.rearrange("o p c -> p o c"))
            wv_sb = res_pool.tile([P, nCc], f32, tag="wv")
            nc.sync.dma_start(out=wv_sb, in_=wv_d.rearrange("o p -> p o"))
            h = res_pool.tile([P, KI, C], bf16, tag="h")

            # ---- phase 1: gate/up matmuls + SiLU*up -> h ----
            with tc.tile_pool(name="ps1", bufs=2, space="PSUM") as ps1:
                for (ig0, ign) in i_groups:
                    iw = ign * P
                    wg_blk = wgu_pool.tile([P, KD, iw], bf16, tag="wg")
                    nc.sync.dma_start(
                        out=wg_blk,
                        in_=wg_d[:, :, ig0 * P: ig0 * P + iw]
                        .rearrange("o p i -> p o i"))
                    wu_blk = wgu_pool.tile([P, KD, iw], bf16, tag="wu")
                    nc.sync.dma_start(
                        out=wu_blk,
                        in_=wu_d[:, :, ig0 * P: ig0 * P + iw]
                        .rearrange("o p i -> p o i"))
                    for j in range(ign):
                        ib = ig0 + j
                        ps_g = [ps1.tile([P, bw], f32, tag=f"g{bi}",
                                         name=f"ps_g{bi}_{ib}")
                                for bi, (b0, bw) in enumerate(c_blocks)]
                        ps_u = [ps1.tile([P, bw], f32, tag=f"u{bi}",
                                         name=f"ps_u{bi}_{ib}")
                                for bi, (b0, bw) in enumerate(c_blocks)]
                        for dk in range(KD):
                            first, last = dk == 0, dk == KD - 1
                            wcol = wg_blk[:, dk, j * P:(j + 1) * P]
                            for bi, (b0, bw) in enumerate(c_blocks):
                                nc.tensor.matmul(
                                    ps_g[bi], lhsT=wcol,
                                    rhs=tok[:, dk, b0:b0 + bw],
                                    start=first, stop=last)
                            wcol = wu_blk[:, dk, j * P:(j + 1) * P]
                            for bi, (b0, bw) in enumerate(c_blocks):
                                nc.tensor.matmul(
                                    ps_u[bi], lhsT=wcol,
                                    rhs=tok[:, dk, b0:b0 + bw],
                                    start=first, stop=last)
                        for bi, (b0, bw) in enumerate(c_blocks):
                            tmp = tmp_pool.tile([P, bw], f32, tag=f"t{bi}")
                            nc.scalar.activation(
                                tmp, ps_g[bi],
                                mybir.ActivationFunctionType.Silu)
                            nc.vector.tensor_mul(
                                h[:, ib, b0:b0 + bw], tmp, ps_u[bi])

            # ---- phase 2: down matmul, scale by combine weight ----
            with tc.tile_pool(name="ps2", bufs=4, space="PSUM") as ps2:
                for (d0, dw) in d_tiles:
                    wd_blk = wd_pool.tile([P, KI, dw], bf16, tag="wd")
                    nc.sync.dma_start(
                        out=wd_blk,
                        in_=wd_d[:, :, d0:d0 + dw].rearrange("o p w -> p o w"))
                    for ci, (c0, cw) in enumerate(c_chunks):
                        ps = ps2.tile([P, dw], f32, tag="y",
                                      name=f"ps_y_{d0}_{ci}")[:cw]
                        for ib in range(KI):
                            nc.tensor.matmul(
                                ps, lhsT=h[:, ib, c0:c0 + cw],
                                rhs=wd_blk[:, ib, :],
                                start=ib == 0, stop=ib == KI - 1)
                        y_sb = y_pool.tile([P, dw], f32, tag="ysb",
                                           name=f"y_sb_{d0}_{ci}")[:cw]
                        nc.vector.tensor_scalar_mul(
                            y_sb, ps, wv_sb[:cw, ci:ci + 1])
                        nc.sync.dma_start(
                            out=y_d[c0:c0 + cw, d0:d0 + dw], in_=y_sb)

    nc.compile()
    return nc


def _prep_core_inputs(t, idx, wvals, C, w_gate_e, w_up_e, w_down_e):
    n = len(idx)
    nCc = math.ceil(C / P)

    tpad = np.zeros((C, DP), np.float32)
    tpad[:n, :D] = t[idx]
    tT = np.ascontiguousarray(tpad.T).reshape(KD, P, C).astype(BF16)

    wg = np.zeros((DP, DP), np.float32)
    wg[:D, :D] = w_gate_e
    wg = wg.reshape(KD, P, DP).astype(BF16)
    wu = np.zeros((DP, DP), np.float32)
    wu[:D, :D] = w_up_e
    wu = wu.reshape(KD, P, DP).astype(BF16)
    wd = np.zeros((DP, D), np.float32)
    wd[:D] = w_down_e
    wd = wd.reshape(KI, P, D).astype(BF16)

    wv = np.zeros((nCc * P,), np.float32)
    wv[:n] = wvals
    wv = wv.reshape(nCc, P)

    return {"tT": tT, "wg": wg, "wu": wu, "wd": wd, "wv": wv}


def moe_forward(x, w_router, w_gate, w_up, w_down, trace=False):
    from concourse.bass_utils import run_bass_kernel_spmd

    x = np.asarray(x)
    t, i1, i2, w1, w2 = _route(x, np.asarray(w_router))
    Ttok = t.shape[0]

    idx_list, wv_list = [], []
    for e in range(E):
        sel1 = i1 == e
        sel2 = i2 == e
        idx = np.nonzero(sel1 | sel2)[0]
        w = np.where(sel1[idx], w1[idx], w2[idx]).astype(np.float32)
        idx_list.append(idx)
        wv_list.append(w)

    C = max(128, math.ceil(max(len(ix) for ix in idx_list) / 64) * 64)

    if C not in _cache:
        _cache[C] = _build_program(C)
    nc = _cache[C]

    wg_f = np.asarray(w_gate, np.float32)
    wu_f = np.asarray(w_up, np.float32)
    wd_f = np.asarray(w_down, np.float32)
    in_maps = [
        _prep_core_inputs(t, idx_list[e], wv_list[e], C,
                          wg_f[e], wu_f[e], wd_f[e])
        for e in range(E)
    ]

    res = run_bass_kernel_spmd(nc, in_maps, list(range(N_CORES)), trace=trace)

    out = np.zeros((Ttok, D), np.float32)
    for e in range(E):
        n = len(idx_list[e])
        out[idx_list[e]] += res.results[e]["y"][:n]

    return out.reshape(x.shape).astype(np.float32), res


def kernel(x, w_router, w_gate, w_up, w_down):
    out, _ = moe_forward(x, w_router, w_gate, w_up, w_down,
                         trace=bool(int(os.environ.get("MOE_TRACE", "0"))))
    return out


# revision 6
# speedup vs baseline: 1.2113x; 1.2113x over previous
"""Trainium2 Bass kernel for a GPT-OSS-style MoE MLP block (top-2 of 8 experts).

Strategy (expert-parallel, full_io):
  - Host computes router softmax + top-2 + renormalized combine weights
    (tiny: [2048, 8]); margins between 2nd/3rd affinities are >=2e-5 for the
    target data, far above fp32 noise, so selection matches the reference.
  - Tokens are dispatched per expert (one expert per NeuronCore), padded to a
    common capacity C; each core runs gate/up matmuls, SiLU*up, down matmul in
    bf16 with fp32 accumulation, scales rows by the combine weight on-device.
  - Host gathers the 8 weighted partial outputs and scatter-adds into [T, D].

Host-side input layouts are chosen so every device DMA reads long
contiguous-per-partition runs:
  tT: [23, 128, C]        bf16  tokens^T, D padded 2880->2944 (=23*128)
  wg: [23, 128, 23, 128]  bf16  [i_blk][d_part][d_chunk][i_in_blk]
  wu: same as wg
  wd: [6, 128, 23, 480]   bf16  [d_tile][i_part][i_chunk][d_in_tile]
  wv: [ceil(C/128), 128]  f32   combine weights (0 in padding slots)
  y : [C, 2880] f32  output (weighted expert output rows)
"""

import math
import os

import ml_dtypes
import numpy as np

T, D, E, TOPK = 2048, 2880, 8, 2
P = 128
DP = 2944  # D and I padded to 23*128
KD = DP // P  # 23 contraction chunks for gate/up
KI = DP // P  # 23 contraction chunks for down
N_CORES = 8
ND = 6  # number of output-D tiles
DW = D // ND  # 480

BF16 = ml_dtypes.bfloat16

_cache = {}


def _route(x, w_router):
    """Host top-2 routing, mirroring the jax reference numerics."""
    t = np.ascontiguousarray(x.reshape(-1, D).astype(np.float32))
    logits = t @ w_router.astype(np.float32)  # [T, E]
    m = logits.max(axis=-1, keepdims=True)
    ex = np.exp(logits - m)
    aff = ex / ex.sum(axis=-1, keepdims=True)
    i1 = aff.argmax(axis=-1)
    a2 = aff.copy()
    a2[np.arange(aff.shape[0]), i1] = -np.inf
    i2 = a2.argmax(axis=-1)
    v1 = aff[np.arange(aff.shape[0]), i1]
    v2 = aff[np.arange(aff.shape[0]), i2]
    s = v1 + v2
    return t, i1, i2, v1 / s, v2 / s


def _blocks(total, max_bs, align):
    """Split `total` into near-equal blocks of size <= max_bs, multiple of
    `align` (except possibly the last)."""
    nb = math.ceil(total / max_bs)
    bs = math.ceil(total / nb / align) * align
    out = []
    off = 0
    while off < total:
        w = min(bs, total - off)
        out.append((off, w))
        off += w
    return out


def _build_program(C):
    import concourse.bacc as bacc
    import concourse.mybir as mybir
    import concourse.tile as tile

    f32 = mybir.dt.float32
    bf16 = mybir.dt.bfloat16

    nCc = math.ceil(C / P)
    c_chunks = _blocks(C, P, 32)          # mm3 output partition chunks
    c_blocks = _blocks(C, 512, 32)        # phase-1 moving free-dim blocks
    d_tiles = [(i * DW, DW) for i in range(ND)]

    nc = bacc.Bacc("TRN2", target_bir_lowering=False, debug=False,
                   num_devices=N_CORES)

    tT_d = nc.dram_tensor("tT", [KD, P, C], bf16, kind="ExternalInput").ap()
    wg_d = nc.dram_tensor("wg", [KI, P, KD, P], bf16,
                          kind="ExternalInput").ap()
    wu_d = nc.dram_tensor("wu", [KI, P, KD, P], bf16,
                          kind="ExternalInput").ap()
    wd_d = nc.dram_tensor("wd", [ND, P, KI, DW], bf16,
                          kind="ExternalInput").ap()
    wv_d = nc.dram_tensor("wv", [nCc, P], f32, kind="ExternalInput").ap()
    y_d = nc.dram_tensor("y", [C, D], f32, kind="ExternalOutput").ap()

    with tile.TileContext(nc) as tc:
        with tc.tile_pool(name="resident", bufs=1) as res_pool, \
             tc.tile_pool(name="wgu", bufs=3) as wgu_pool, \
             tc.tile_pool(name="wd", bufs=2) as wd_pool, \
             tc.tile_pool(name="tmp", bufs=2) as tmp_pool, \
             tc.tile_pool(name="yev", bufs=3) as y_pool:

            h = res_pool.tile([P, KI, C], bf16, tag="h")
            tok = res_pool.tile([P, KD, C], bf16, tag="tok")

            # ---- phase 1: gate/up matmuls + SiLU*up -> h ----
            with tc.tile_pool(name="ps1", bufs=2, space="PSUM") as ps1:
                for ib in range(KI):
                    wg_blk = wgu_pool.tile([P, KD, P], bf16, tag="wg",
                                           name=f"wg_blk_{ib}")
                    nc.sync.dma_start(out=wg_blk, in_=wg_d[ib])
                    wu_blk = wgu_pool.tile([P, KD, P], bf16, tag="wu",
                                           name=f"wu_blk_{ib}")
                    nc.sync.dma_start(out=wu_blk, in_=wu_d[ib])
                    if ib == 0:
                        # token chunks land while the first weight block loads
                        for dk in range(KD):
                            nc.sync.dma_start(out=tok[:, dk, :], in_=tT_d[dk])
                    ps_g = [ps1.tile([P, bw], f32, tag=f"g{bi}",
                                     name=f"ps_g{bi}_{ib}")
                            for bi, (b0, bw) in enumerate(c_blocks)]
                    ps_u = [ps1.tile([P, bw], f32, tag=f"u{bi}",
                                     name=f"ps_u{bi}_{ib}")
                            for bi, (b0, bw) in enumerate(c_blocks)]
                    for dk in range(KD):
                        first, last = dk == 0, dk == KD - 1
                        for bi, (b0, bw) in enumerate(c_blocks):
                            nc.tensor.matmul(
                                ps_g[bi], lhsT=wg_blk[:, dk, :],
                                rhs=tok[:, dk, b0:b0 + bw],
                                start=first, stop=last)
                        for bi, (b0, bw) in enumerate(c_blocks):
                            nc.tensor.matmul(
                                ps_u[bi], lhsT=wu_blk[:, dk, :],
                                rhs=tok[:, dk, b0:b0 + bw],
                                start=first, stop=last)
                    for bi, (b0, bw) in enumerate(c_blocks):
                        tmp = tmp_pool.tile([P, bw], f32, tag=f"t{bi}",
                                            name=f"tmp{bi}_{ib}")
                        nc.scalar.activation(
                            tmp, ps_g[bi], mybir.ActivationFunctionType.Silu)
                        nc.vector.tensor_mul(
                            h[:, ib, b0:b0 + bw], tmp, ps_u[bi])

            wv_sb = res_pool.tile([P, nCc], f32, tag="wv")
            nc.sync.dma_start(out=wv_sb, in_=wv_d.rearrange("o p -> p o"))

            # ---- phase 2: down matmul, scale by combine weight ----
            with tc.tile_pool(name="ps2", bufs=4, space="PSUM") as ps2:
                for dti, (d0, dw) in enumerate(d_tiles):
                    wd_blk = wd_pool.tile([P, KI, dw], bf16, tag="wd",
                                          name=f"wd_blk_{dti}")
                    nc.sync.dma_start(out=wd_blk, in_=wd_d[dti])
                    for ci, (c0, cw) in enumerate(c_chunks):
                        ps = ps2.tile([P, dw], f32, tag="y",
                                      name=f"ps_y_{d0}_{ci}")[:cw]
                        for ib in range(KI):
                            nc.tensor.matmul(
                                ps, lhsT=h[:, ib, c0:c0 + cw],
                                rhs=wd_blk[:, ib, :],
                                start=ib == 0, stop=ib == KI - 1)
                        y_sb = y_pool.tile([P, dw], f32, tag="ysb",
                                           name=f"y_sb_{d0}_{ci}")[:cw]
                        nc.vector.tensor_scalar_mul(
                            y_sb, ps, wv_sb[:cw, ci:ci + 1])
                        nc.sync.dma_start(
                            out=y_d[c0:c0 + cw, d0:d0 + dw], in_=y_sb)

    nc.compile()
    return nc


def _prep_core_inputs(t, idx, wvals, C, w_gate_e, w_up_e, w_down_e):
    n = len(idx)
    nCc = math.ceil(C / P)

    tpad = np.zeros((C, DP), np.float32)
    tpad[:n, :D] = t[idx]
    tT = np.ascontiguousarray(tpad.T).reshape(KD, P, C).astype(BF16)

    # wg/wu: [D, I] -> pad to [DP, DP] -> [i_blk, p, d_chunk, i_in_blk]
    wg = np.zeros((DP, DP), np.float32)
    wg[:D, :D] = w_gate_e
    # [dk, dp, ik, ip] -> [ik, dp, dk, ip]
    wg = np.ascontiguousarray(
        wg.reshape(KD, P, KI, P).transpose(2, 1, 0, 3)).astype(BF16)
    wu = np.zeros((DP, DP), np.float32)
    wu[:D, :D] = w_up_e
    wu = np.ascontiguousarray(
        wu.reshape(KD, P, KI, P).transpose(2, 1, 0, 3)).astype(BF16)

    # wd: [I, D] -> pad I -> [d_tile, i_part, i_chunk, d_in_tile]
    wd = np.zeros((DP, D), np.float32)
    wd[:D] = w_down_e
    wd = np.ascontiguousarray(
        wd.reshape(KI, P, ND, DW).transpose(2, 1, 0, 3)).astype(BF16)

    wv = np.zeros((nCc * P,), np.float32)
    wv[:n] = wvals
    wv = wv.reshape(nCc, P)

    return {"tT": tT, "wg": wg, "wu": wu, "wd": wd, "wv": wv}


def moe_forward(x, w_router, w_gate, w_up, w_down, trace=False):
    from concourse.bass_utils import run_bass_kernel_spmd

    x = np.asarray(x)
    t, i1, i2, w1, w2 = _route(x, np.asarray(w_router))
    Ttok = t.shape[0]

    idx_list, wv_list = [], []
    for e in range(E):
        sel1 = i1 == e
        sel2 = i2 == e
        idx = np.nonzero(sel1 | sel2)[0]
        w = np.where(sel1[idx], w1[idx], w2[idx]).astype(np.float32)
        idx_list.append(idx)
        wv_list.append(w)

    C = max(128, math.ceil(max(len(ix) for ix in idx_list) / 64) * 64)

    if C not in _cache:
        _cache[C] = _build_program(C)
    nc = _cache[C]

    wg_f = np.asarray(w_gate, np.float32)
    wu_f = np.asarray(w_up, np.float32)
    wd_f = np.asarray(w_down, np.float32)
    in_maps = [
        _prep_core_inputs(t, idx_list[e], wv_list[e], C,
                          wg_f[e], wu_f[e], wd_f[e])
        for e in range(E)
    ]

    res = run_bass_kernel_spmd(nc, in_maps, list(range(N_CORES)), trace=trace)

    out = np.zeros((Ttok, D), np.float32)
    for e in range(E):
        n = len(idx_list[e])
        out[idx_list[e]] += res.results[e]["y"][:n]

    return out.reshape(x.shape).astype(np.float32), res


def kernel(x, w_router, w_gate, w_up, w_down):
    out, _ = moe_forward(x, w_router, w_gate, w_up, w_down,
                         trace=bool(int(os.environ.get("MOE_TRACE", "0"))))
    return out


# revision 7
# speedup vs baseline: 1.2286x; 1.0143x over previous
"""Trainium2 Bass kernel for a GPT-OSS-style MoE MLP block (top-2 of 8 experts).

Strategy (expert-parallel, full_io):
  - Host computes router softmax + top-2 + renormalized combine weights
    (tiny: [2048, 8]); margins between 2nd/3rd affinities are >=2e-5 for the
    target data, far above fp32 noise, so selection matches the reference.
  - Tokens are dispatched per expert (one expert per NeuronCore), padded to a
    common capacity C; each core runs gate/up matmuls, then h = combine_w *
    SiLU(gate) * up (bf16, fp32 accumulation), then the down matmul producing
    the (transposed) weighted expert output yT.
  - Host gathers the 8 partial outputs and scatter-adds into [T, D].

Host-side input layouts are chosen so every device DMA reads long
contiguous-per-partition runs:
  tT : [23, 128, C]        bf16  tokens^T, D padded 2880->2944 (=23*128)
  wg : [23, 128, 23, 128]  bf16  [i_blk][d_part][d_chunk][i_in_blk]
  wu : same as wg
  wd : [23, 128, 23, 128]  bf16  [d_chunk][i_part][i_chunk][d_in_chunk],
                                 D padded to 2944
  wvr: [128, C]            f32   combine weights replicated per partition
  yT : [23, 128, C] f32  output chunk-transposed: yT[dc,dp,c] = y[c, dc*128+dp]
"""

import math
import os

import ml_dtypes
import numpy as np

T, D, E, TOPK = 2048, 2880, 8, 2
P = 128
DP = 2944  # D and I padded to 23*128
KD = DP // P  # 23 contraction chunks for gate/up
KI = DP // P  # 23 contraction chunks for down
KO = DP // P  # 23 output-D chunks (padded)
N_CORES = 8

BF16 = ml_dtypes.bfloat16

_cache = {}


def _route(x, w_router):
    """Host top-2 routing, mirroring the jax reference numerics."""
    t = np.ascontiguousarray(x.reshape(-1, D).astype(np.float32))
    logits = t @ w_router.astype(np.float32)  # [T, E]
    m = logits.max(axis=-1, keepdims=True)
    ex = np.exp(logits - m)
    aff = ex / ex.sum(axis=-1, keepdims=True)
    i1 = aff.argmax(axis=-1)
    a2 = aff.copy()
    a2[np.arange(aff.shape[0]), i1] = -np.inf
    i2 = a2.argmax(axis=-1)
    v1 = aff[np.arange(aff.shape[0]), i1]
    v2 = aff[np.arange(aff.shape[0]), i2]
    s = v1 + v2
    return t, i1, i2, v1 / s, v2 / s


def _blocks(total, max_bs, align):
    """Split `total` into near-equal blocks of size <= max_bs, multiple of
    `align` (except possibly the last)."""
    nb = math.ceil(total / max_bs)
    bs = math.ceil(total / nb / align) * align
    out = []
    off = 0
    while off < total:
        w = min(bs, total - off)
        out.append((off, w))
        off += w
    return out


def _build_program(C):
    import concourse.bacc as bacc
    import concourse.mybir as mybir
    import concourse.tile as tile

    f32 = mybir.dt.float32
    bf16 = mybir.dt.bfloat16

    c_blocks = _blocks(C, 512, 32)  # moving free-dim blocks (both phases)

    nc = bacc.Bacc("TRN2", target_bir_lowering=False, debug=False,
                   num_devices=N_CORES)

    tT_d = nc.dram_tensor("tT", [KD, P, C], bf16, kind="ExternalInput").ap()
    wg_d = nc.dram_tensor("wg", [KI, P, KD, P], bf16,
                          kind="ExternalInput").ap()
    wu_d = nc.dram_tensor("wu", [KI, P, KD, P], bf16,
                          kind="ExternalInput").ap()
    wd_d = nc.dram_tensor("wd", [KO, P, KI, P], bf16,
                          kind="ExternalInput").ap()
    wvr_d = nc.dram_tensor("wvr", [P, C], f32, kind="ExternalInput").ap()
    yT_d = nc.dram_tensor("yT", [KO, P, C], f32, kind="ExternalOutput").ap()

    with tile.TileContext(nc) as tc:
        with tc.tile_pool(name="resident", bufs=1) as res_pool, \
             tc.tile_pool(name="wgu", bufs=3) as wgu_pool, \
             tc.tile_pool(name="wdp", bufs=3) as wd_pool, \
             tc.tile_pool(name="tmp", bufs=2) as tmp_pool, \
             tc.tile_pool(name="yev", bufs=3) as y_pool:

            h = res_pool.tile([P, KI, C], bf16, tag="h")
            tok = res_pool.tile([P, KD, C], bf16, tag="tok")
            wvr = res_pool.tile([P, C], f32, tag="wvr")

            # ---- phase 1: gate/up matmuls, h = wv * SiLU(gate) * up ----
            with tc.tile_pool(name="ps1", bufs=2, space="PSUM") as ps1:
                for ib in range(KI):
                    wg_blk = wgu_pool.tile([P, KD, P], bf16, tag="wg",
                                           name=f"wg_blk_{ib}")
                    nc.sync.dma_start(out=wg_blk, in_=wg_d[ib])
                    if ib == 0:
                        # token chunks land while the first weight block loads
                        for dk in range(KD):
                            nc.sync.dma_start(out=tok[:, dk, :], in_=tT_d[dk])
                    wu_blk = wgu_pool.tile([P, KD, P], bf16, tag="wu",
                                           name=f"wu_blk_{ib}")
                    nc.sync.dma_start(out=wu_blk, in_=wu_d[ib])
                    if ib == 0:
                        nc.sync.dma_start(out=wvr, in_=wvr_d)
                    ps_g = [ps1.tile([P, bw], f32, tag=f"g{bi}",
                                     name=f"ps_g{bi}_{ib}")
                            for bi, (b0, bw) in enumerate(c_blocks)]
                    ps_u = [ps1.tile([P, bw], f32, tag=f"u{bi}",
                                     name=f"ps_u{bi}_{ib}")
                            for bi, (b0, bw) in enumerate(c_blocks)]
                    for dk in range(KD):
                        first, last = dk == 0, dk == KD - 1
                        for bi, (b0, bw) in enumerate(c_blocks):
                            nc.tensor.matmul(
                                ps_g[bi], lhsT=wg_blk[:, dk, :],
                                rhs=tok[:, dk, b0:b0 + bw],
                                start=first, stop=last)
                    for dk in range(KD):
                        first, last = dk == 0, dk == KD - 1
                        for bi, (b0, bw) in enumerate(c_blocks):
                            nc.tensor.matmul(
                                ps_u[bi], lhsT=wu_blk[:, dk, :],
                                rhs=tok[:, dk, b0:b0 + bw],
                                start=first, stop=last)
                    for bi, (b0, bw) in enumerate(c_blocks):
                        tmp = tmp_pool.tile([P, bw], f32, tag=f"t{bi}",
                                            name=f"tmp{bi}_{ib}")
                        nc.scalar.activation(
                            tmp, ps_g[bi], mybir.ActivationFunctionType.Silu)
                        tmp2 = tmp_pool.tile([P, bw], f32, tag=f"t2{bi}",
                                             name=f"tmp2_{bi}_{ib}")
                        nc.vector.tensor_mul(tmp2, tmp, ps_u[bi])
                        nc.vector.tensor_mul(
                            h[:, ib, b0:b0 + bw], tmp2,
                            wvr[:, b0:b0 + bw])

            # ---- phase 2: down matmul -> yT ----
            with tc.tile_pool(name="ps2", bufs=2, space="PSUM") as ps2:
                for dc in range(KO):
                    wd_blk = wd_pool.tile([P, KI, P], bf16, tag="wd",
                                          name=f"wd_blk_{dc}")
                    nc.sync.dma_start(out=wd_blk, in_=wd_d[dc])
                    ps_y = [ps2.tile([P, bw], f32, tag=f"y{bi}",
                                     name=f"ps_y{bi}_{dc}")
                            for bi, (b0, bw) in enumerate(c_blocks)]
                    for ib in range(KI):
                        first, last = ib == 0, ib == KI - 1
                        for bi, (b0, bw) in enumerate(c_blocks):
                            nc.tensor.matmul(
                                ps_y[bi], lhsT=wd_blk[:, ib, :],
                                rhs=h[:, ib, b0:b0 + bw],
                                start=first, stop=last)
                    y_sb = y_pool.tile([P, C], f32, tag="ysb",
                                       name=f"y_sb_{dc}")
                    for bi, (b0, bw) in enumerate(c_blocks):
                        nc.scalar.copy(y_sb[:, b0:b0 + bw], ps_y[bi])
                    nc.sync.dma_start(out=yT_d[dc], in_=y_sb)

    nc.compile()
    return nc


def _prep_core_inputs(t, idx, wvals, C, w_gate_e, w_up_e, w_down_e):
    n = len(idx)

    tpad = np.zeros((C, DP), np.float32)
    tpad[:n, :D] = t[idx]
    tT = np.ascontiguousarray(tpad.T).reshape(KD, P, C).astype(BF16)

    # wg/wu: [D, I] -> pad to [DP, DP]; [dk, dp, ik, ip] -> [ik, dp, dk, ip]
    wg = np.zeros((DP, DP), np.float32)
    wg[:D, :D] = w_gate_e
    wg = np.ascontiguousarray(
        wg.reshape(KD, P, KI, P).transpose(2, 1, 0, 3)).astype(BF16)
    wu = np.zeros((DP, DP), np.float32)
    wu[:D, :D] = w_up_e
    wu = np.ascontiguousarray(
        wu.reshape(KD, P, KI, P).transpose(2, 1, 0, 3)).astype(BF16)

    # wd: [I, D] -> pad both to DP; [ik, ip, dc, dp] -> [dc, ip, ik, dp]
    wd = np.zeros((DP, DP), np.float32)
    wd[:D, :D] = w_down_e
    wd = np.ascontiguousarray(
        wd.reshape(KI, P, KO, P).transpose(2, 1, 0, 3)).astype(BF16)

    wv = np.zeros((C,), np.float32)
    wv[:n] = wvals
    wvr = np.ascontiguousarray(np.broadcast_to(wv, (P, C)))

    return {"tT": tT, "wg": wg, "wu": wu, "wd": wd, "wvr": wvr}


def moe_forward(x, w_router, w_gate, w_up, w_down, trace=False):
    from concourse.bass_utils import run_bass_kernel_spmd

    x = np.asarray(x)
    t, i1, i2, w1, w2 = _route(x, np.asarray(w_router))
    Ttok = t.shape[0]

    idx_list, wv_list = [], []
    for e in range(E):
        sel1 = i1 == e
        sel2 = i2 == e
        idx = np.nonzero(sel1 | sel2)[0]
        w = np.where(sel1[idx], w1[idx], w2[idx]).astype(np.float32)
        idx_list.append(idx)
        wv_list.append(w)

    C = max(128, math.ceil(max(len(ix) for ix in idx_list) / 64) * 64)

    if C not in _cache:
        _cache[C] = _build_program(C)
    nc = _cache[C]

    wg_f = np.asarray(w_gate, np.float32)
    wu_f = np.asarray(w_up, np.float32)
    wd_f = np.asarray(w_down, np.float32)
    in_maps = [
        _prep_core_inputs(t, idx_list[e], wv_list[e], C,
                          wg_f[e], wu_f[e], wd_f[e])
        for e in range(E)
    ]

    res = run_bass_kernel_spmd(nc, in_maps, list(range(N_CORES)), trace=trace)

    out = np.zeros((Ttok, D), np.float32)
    for e in range(E):
        n = len(idx_list[e])
        yT = res.results[e]["yT"].reshape(DP, C)  # [dc*128+dp, c]
        out[idx_list[e]] += yT[:D, :n].T

    return out.reshape(x.shape).astype(np.float32), res


def kernel(x, w_router, w_gate, w_up, w_down):
    out, _ = moe_forward(x, w_router, w_gate, w_up, w_down,
                         trace=bool(int(os.environ.get("MOE_TRACE", "0"))))
    return out
